# revision 57
# baseline (speedup 1.0000x reference)
"""Trainium2 Bass kernel for CRF negative log-likelihood (nn_CRF) — v3.

Strategy:
  - data-parallel over batch: 8 cores x 16 sequences each.
  - forward algorithm in the exp domain: the fwd chain (alpha, t=0..127) and
    the bwd chain (beta, t=255..128) are MERGED into one 128-step scan over a
    block-diagonal bf16 stationary G (Etil at rows/cols 0..51, Etil^T at
    64..115; blocks at 0/64 keep engine partition bases 32-aligned).  Each
    step is ONE bf16 matmul (PE) + ONE elementwise multiply (DVE); the
    serial PE->DVE->PE round trip (~435ns) is the latency floor, so all other
    work lives on ACT/GPSIMD/DMA:
      * emissions D2 (128, HALF, BL) bf16: fwd emissions on rows 0..51, the
        time-reversed bwd emissions on rows 64..115; exp(-C0) rescale and the
        absorbing-STOP mask gate folded in via per-partition ACT bias plus
        DMA accumulate (accum_op=add) of host-scaled mask rows — no vector
        engine involvement at all.
      * gold-score emission gather: host-built one-hot * feats on GPSIMD.
      * host does integer prep only (one-hots, pair/end counts); all float
        math on feats/transitions happens on device.
  - one renorm at k=64 bounds fp32/bf16 range; raw colsums staged out and the
    host adds the logs back (plus C0 * length per sequence).
"""

import numpy as np

TAG = 52
START, STOP = TAG - 2, TAG - 1
B, S = 128, 256
NCORES = 8
BL = B // NCORES            # 16 sequences per core
HALF = S // 2               # 128 steps per direction
C0 = 4.9                    # constant per-step rescale (nats)
MGATE = 64.0                # mask gate constant (exp(-64) == 0 in fp32)
M32 = (S * BL) // 128       # 32 free rows for the (128, M32, TAG) gold layout
ROWB = 64                   # partition offset of the bwd block
GROUPS = ((0, 8), (8, 32), (32, 64), (64, HALF))   # emission build chunk groups
NFOLD = 32          # steps whose mask gate the host folds into featsT2

# packed "smalls" layout (columns in a single (128, SMW) f32 tensor)
C_TR = 0            # [0:52]   rows 0:52  transitions (STOP,STOP pre-patched)
C_TT = 52           # [52:104] rows 0:52  transitions TRANSPOSED (same patch)
C_CC = 104          # [104:106]           colcs (sgate unused now, bias)
C_HS = 106          # [106:108]           Hsum pattern (f32 -> bf16 copy)
C_HB = 108          # [108:236] rows 0:2  Hbc
C_F0 = 236          # [236:252] rows 0:52 feats[:, 0, :].T
C_CP = 252          # [252:304] rows 0:52 pair counts
C_CE = 304          # [304:305] rows 0:52 end counts
C_B0 = 305          # [305:306] rows 0:52 init bias (top bias + trans[START,:])
SMW = 306

_CACHE: dict = {}


def _build_nc(debug: bool = False):
    import concourse.bass as bass
    import concourse.mybir as mybir
    import concourse.tile as tile
    from concourse import bacc

    f32 = mybir.dt.float32
    bf16 = mybir.dt.bfloat16
    AL = mybir.AluOpType
    EXP = mybir.ActivationFunctionType.Exp

    nc = bacc.Bacc("TRN2", target_bir_lowering=False, debug=debug)

    # ---- external inputs (per-core shards, host-marshalled layouts) ----
    featsT2 = nc.dram_tensor("featsT2", (128, HALF, BL), f32, kind="ExternalInput")
    mgate = nc.dram_tensor("mgate", (2, HALF, BL), f32, kind="ExternalInput")
    smalls = nc.dram_tensor("smalls", (128, SMW), f32, kind="ExternalInput")
    onebl = nc.dram_tensor("onebl", (1, BL), bf16, kind="ExternalInput")
    fny = nc.dram_tensor("fny", (128, M32, 2 * TAG), f32, kind="ExternalInput")

    # ---- external outputs ----
    # out_scan: [0, 0:BL] = midpoint colsum (no renorm: the C0 rescale keeps
    # the state within ~e^{+-10} over each 128-step half chain)
    out_scan = nc.dram_tensor("out_scan", (1, BL), f32, kind="ExternalOutput")
    # out_gold: [0,0] = emit sum; col1 = trans*cnt partials; col2 = end partials
    out_gold = nc.dram_tensor("out_gold", (128, 4), f32, kind="ExternalOutput")

    with tile.TileContext(nc) as tc:
        with (
            tc.tile_pool(name="persist", bufs=1) as persist,
            tc.tile_pool(name="chunks", bufs=1) as chunks,
            tc.tile_pool(name="state", bufs=3) as statep,
            tc.tile_pool(name="small", bufs=2) as small,
            tc.tile_pool(name="gold", bufs=1) as goldp,
            tc.tile_pool(name="psum", bufs=1, space="PSUM") as psum,
            tc.tile_pool(name="psumg", bufs=1, space="PSUM") as psumg,
        ):
            # ---- ACT activation-table prefetch: dummy exp at t=0 ----
            junk = small.tile([1, 1], f32, name="junk", tag="junk")
            nc.gpsimd.memset(junk, 0.0)
            junk2 = small.tile([1, 1], f32, name="junk2", tag="junk2")
            nc.scalar.activation(out=junk2, in_=junk, func=EXP)

            # ---- head DMAs in dependency-priority order ----
            SM = persist.tile([128, SMW], f32, name="SM", tag="SM")
            nc.sync.dma_start(out=SM, in_=smalls[:, :])
            tr_sb = SM[0:TAG, C_TR : C_TR + TAG]
            colc = SM[:, C_CC : C_CC + 2]

            fts = {}
            for s0, s1 in GROUPS:
                fts[s0] = chunks.tile(
                    [128, s1 - s0, BL], f32, name=f"ft{s0}", tag=f"ft{s0}"
                )
            nc.sync.dma_start(out=fts[0], in_=featsT2[:, 0 : GROUPS[0][1], :])

            # ================= transitions -> G blockdiag (direct ACT) =======
            # (small pad keeps G off a slow SBUF placement; measured best)
            persist.tile([128, 8], f32, name="padg", tag="padg")
            G = persist.tile([128, 128], bf16, name="G", tag="G")
            nc.gpsimd.memset(G, 0.0)
            nc.scalar.activation(
                out=G[0:TAG, 0:TAG], in_=SM[0:TAG, C_TR : C_TR + TAG], func=EXP
            )
            nc.scalar.activation(
                out=G[ROWB : ROWB + TAG, ROWB : ROWB + TAG],
                in_=SM[ROWB : ROWB + TAG, C_TT : C_TT + TAG],
                func=EXP,
            )

            # ================= scan state init =================
            # V0 top = exp(f0 + bias + trans[START, :]) (bias col C_B0);
            # bottom = onehot(STOP) via tiny DMA (arbitrary partition base)
            V = statep.tile([128, BL], bf16, name="V0", tag="V")
            nc.gpsimd.memset(V, 0.0)
            nc.scalar.activation(
                out=V[0:TAG, :],
                in_=SM[0:TAG, C_F0 : C_F0 + BL],
                func=EXP,
                bias=SM[0:TAG, C_B0 : C_B0 + 1],
            )
            nc.sync.dma_start(
                out=V[ROWB + STOP : ROWB + STOP + 1, :], in_=onebl[:, :]
            )

            # ================= emission tensor D2 (128, HALF, BL) ============
            # no memset: the exps cover the dead mid rows too (ft rows are
            # host-zeroed there and the bias is -MGATE, so exp gives 0); the
            # one ACT per group covers all 128 rows via the per-partition bias
            D2 = persist.tile([128, HALF, BL], bf16, name="D2", tag="D2")

            def emit_exps(s0, s1):
                nc.scalar.activation(
                    out=D2[:, s0:s1, :],
                    in_=fts[s0],
                    func=EXP,
                    bias=colc[:, 1:2],
                )

            def emit_gate(s0, s1):
                # mask gate via DMA accumulate: rows 64..114 += m*MGATE,
                # row 115 (STOP) += (1-m)*MGATE
                n = s1 - s0
                ft = fts[s0]
                srcp = bass.AP(
                    tensor=mgate,
                    offset=s0 * BL,
                    ap=[[0, TAG - 1], [BL, n], [1, BL]],
                )
                nc.gpsimd.dma_start(
                    out=ft[ROWB : ROWB + TAG - 1, :, :], in_=srcp, accum_op=AL.add
                )
                srcn = bass.AP(
                    tensor=mgate,
                    offset=HALF * BL + s0 * BL,
                    ap=[[0, 1], [BL, n], [1, BL]],
                )
                nc.gpsimd.dma_start(
                    out=ft[ROWB + TAG - 1 : ROWB + TAG, :, :],
                    in_=srcn,
                    accum_op=AL.add,
                )

            # groups below NFOLD: the mask gate is host-folded into featsT2
            # (cold-start latency), so they go straight to exp
            emit_exps(*GROUPS[0])
            # layout ballast: restores the SBUF tile offsets of the renorm
            # build (removing these measurably slowed every engine op ~20%)
            Hbcb = persist.tile([2, 128], bf16, name="Hbcb", tag="Hbcb")
            nc.gpsimd.memset(Hbcb, 0.0)
            Hsum = persist.tile([128, 2], bf16, name="Hsum", tag="Hsum")
            nc.vector.tensor_copy(Hsum, SM[:, C_HS : C_HS + 2])
            ones52 = persist.tile([TAG, 1], bf16, name="ones52", tag="ones52")
            nc.gpsimd.memset(ones52, 1.0)

            for s0, s1 in GROUPS[1:]:
                nc.sync.dma_start(out=fts[s0], in_=featsT2[:, s0:s1, :])
                if s0 >= NFOLD:
                    emit_gate(s0, s1)
                emit_exps(s0, s1)

            # ================= interleaved fwd/bwd scan =================
            stage_sc = persist.tile([1, BL], f32, name="stage_sc", tag="ssc")

            KLIVE = ROWB + TAG   # contraction rows 0..115; 116..127 are dead
            for k in range(1, HALF + 1):
                ps = psum.tile([128, BL], f32, name="ps", tag="ps", bufs=2)
                nc.tensor.matmul(
                    ps, G[0:KLIVE, :], V[0:KLIVE, :], start=True, stop=True
                )
                Vn = statep.tile([128, BL], bf16, name="Vn", tag="V")
                nc.vector.tensor_tensor(
                    out=Vn, in0=D2[:, k - 1, :], in1=ps, op=AL.mult
                )
                V = Vn

            # ================= gold (GPSIMD, off the scan path) ==============
            FNY = goldp.tile([128, M32, 2 * TAG], f32, name="FNY", tag="FNY")
            nc.sync.dma_start(out=FNY, in_=fny[:, :, :])
            stage_gold = goldp.tile([128, 4], f32, name="stage_gold", tag="sg")
            nc.gpsimd.memset(stage_gold, 0.0)
            scrap = goldp.tile([128, M32, TAG], f32, name="scrap", tag="scrap")
            nc.gpsimd.tensor_tensor(
                out=scrap,
                in0=FNY[:, :, 0:TAG],
                in1=FNY[:, :, TAG : 2 * TAG],
                op=AL.mult,
            )
            nc.gpsimd.tensor_reduce(
                out=stage_gold[0:1, 0:1],
                in_=scrap,
                axis=mybir.AxisListType.XYZWC,
                op=AL.add,
            )

            # ================= tail: Z = (Etil^T alpha_127) . U_128 ===========
            # step 128's PSUM top rows already hold Etil^T alpha_127; multiply
            # them against the bottom half of the final state (mixed partition
            # bases 0 and 64, both 32-aligned)
            P = small.tile([TAG, BL], bf16, name="P", tag="P")
            nc.vector.tensor_tensor(
                out=P, in0=V[ROWB : ROWB + TAG, :], in1=ps[0:TAG, :], op=AL.mult
            )
            ps_c = psumg.tile([1, BL], f32, name="ps_c", tag="ps_c")
            nc.tensor.matmul(ps_c, ones52, P, start=True, stop=True)
            nc.vector.tensor_copy(stage_sc, ps_c)
            nc.sync.dma_start(out=out_scan[:, :], in_=stage_sc)

            # ================= gold tail: transition contractions (GPSIMD) ====
            scrap2 = small.tile([TAG, TAG], f32, name="scrap2", tag="scrap2")
            nc.gpsimd.tensor_tensor(
                out=scrap2,
                in0=SM[0:TAG, C_CP : C_CP + TAG],
                in1=tr_sb,
                op=AL.mult,
            )
            nc.gpsimd.tensor_reduce(
                out=stage_gold[0:1, 1:2],
                in_=scrap2,
                axis=mybir.AxisListType.XYZWC,
                op=AL.add,
            )
            scrap3 = small.tile([TAG, 1], f32, name="scrap3", tag="scrap3")
            nc.gpsimd.tensor_tensor(
                out=scrap3,
                in0=SM[0:TAG, C_CE : C_CE + 1],
                in1=tr_sb[:, STOP : STOP + 1],
                op=AL.mult,
            )
            nc.gpsimd.tensor_reduce(
                out=stage_gold[0:1, 2:3],
                in_=scrap3,
                axis=mybir.AxisListType.XYZWC,
                op=AL.add,
            )
            nc.sync.dma_start(out=out_gold[:, :], in_=stage_gold)

    nc.compile()
    return nc


def _prep_core_inputs(feats, transitions, mask, tags, core):
    """Host marshalling of the core's batch shard: layout + integer prep."""
    import ml_dtypes

    f32 = np.float32
    bf = ml_dtypes.bfloat16
    sl = slice(core * BL, (core + 1) * BL)
    f = np.ascontiguousarray(feats[sl]).astype(f32, copy=False)   # (BL,S,T)
    m = mask[sl].astype(f32)                                      # (BL,S)
    tg = tags[sl].astype(np.int64)                                # (BL,S)

    fT = f.transpose(2, 1, 0)                                     # (T,S,BL)
    ft2 = np.zeros((128, HALF, BL), f32)
    ft2[0:TAG, 0:127, :] = fT[:, 1:128, :]
    ft2[STOP, 0:127, :] = 0.0
    ft2[0:TAG, 127, :] = -200.0                                   # dead fwd slot
    ft2[ROWB : ROWB + TAG, :, :] = fT[:, 255:127:-1, :]
    ft2[ROWB + STOP, :, :] = 0.0

    mtb = np.ascontiguousarray(m.T[255:127:-1, :])                # (HALF,BL)
    mg = np.stack([mtb * MGATE, (1.0 - mtb) * MGATE]).astype(f32)  # (2,HALF,BL)
    # host-fold the gate into the first NFOLD steps (cold-start latency):
    # rows 64..114 += m*MGATE, row 115 += (1-m)*MGATE
    ft2[ROWB : ROWB + TAG - 1, 0:NFOLD, :] += mg[0, None, 0:NFOLD, :]
    ft2[ROWB + TAG - 1, 0:NFOLD, :] += mg[1, 0:NFOLD, :]

    smalls = np.zeros((128, SMW), f32)
    trp = transitions.astype(f32).copy()
    trp[STOP, STOP] = 0.0
    smalls[0:TAG, C_TR : C_TR + TAG] = trp
    smalls[ROWB : ROWB + TAG, C_TT : C_TT + TAG] = trp.T   # rows 64:116: ACT
    # reads/writes must share a 32-aligned partition base with their output
    # colcs: col0 unused, col1 = per-partition exp bias; dead mid rows get
    # -MGATE so the widened exps write exact zeros there (no D2 memset)
    smalls[:, C_CC + 1] = -MGATE
    smalls[0:TAG, C_CC + 1] = -C0
    smalls[STOP, C_CC + 1] = -MGATE
    smalls[ROWB : ROWB + TAG, C_CC + 1] = -(MGATE + C0)
    smalls[ROWB + STOP, C_CC + 1] = -MGATE   # pairs with the (1-m) gate row
    smalls[0:TAG, C_HS] = 1.0                 # Hsum col0: fwd half
    smalls[ROWB : ROWB + TAG, C_HS + 1] = 1.0
    smalls[0, C_HB : C_HB + TAG] = 1.0        # Hbc row0 -> fwd rows
    smalls[1, C_HB + ROWB : C_HB + ROWB + TAG] = 1.0
    f0 = f[:, 0, :].T.copy()
    f0[STOP] = 0.0
    smalls[0:TAG, C_F0 : C_F0 + BL] = f0
    # init bias: top exp bias + trans[START, :] (folds the alpha_0 init
    # multiply by exp(trans[START, j]) into the one ACT that builds V0)
    smalls[0:TAG, C_B0] = smalls[0:TAG, C_CC + 1] + trp[START, :]

    prev = np.concatenate([np.full((BL, 1), START, np.int64), tg[:, :-1]], axis=1)
    msk = m > 0
    cntp = np.zeros((TAG, TAG), f32)
    np.add.at(cntp, (prev[msk], tg[msk]), 1.0)
    smalls[0:TAG, C_CP : C_CP + TAG] = cntp
    lengths = m.astype(np.int64).sum(axis=1)
    end_ids = np.take_along_axis(tg, (lengths - 1)[:, None], axis=1)[:, 0]
    cnte = np.zeros((TAG,), f32)
    np.add.at(cnte, end_ids, 1.0)
    smalls[0:TAG, C_CE] = cnte

    featsN = np.ascontiguousarray(f.reshape(BL * S, TAG)).reshape(128, M32, TAG)
    tags_m = np.where(m > 0, tg, -1)
    yhot = (
        (tags_m.reshape(BL * S, 1) == np.arange(TAG)[None, :])
        .astype(f32)
        .reshape(128, M32, TAG)
    )
    fny = np.concatenate([featsN, yhot], axis=2)                  # (128,M32,104)

    return {
        "featsT2": ft2,
        "mgate": mg,
        "smalls": smalls,
        "onebl": np.ones((1, BL), bf),
        "fny": np.ascontiguousarray(fny),
    }


def _combine(results, mask):
    """Host-side unshard: logs of staged scales + partial sums -> scalar."""
    lengths = np.asarray(mask).astype(np.int64).sum(axis=1)       # (B,)
    fwd = np.float64(0.0)
    gold = np.float64(0.0)
    for core, res in enumerate(results):
        sc = res["out_scan"].astype(np.float64)                   # (1, BL)
        gl = res["out_gold"].astype(np.float64)                   # (128, 4)
        ln = np.log(sc[0, 0:BL])
        lens = lengths[core * BL : (core + 1) * BL].astype(np.float64)
        fwd += (ln + C0 * lens).sum()
        gold += gl[0, 0] + gl[0, 1] + gl[0, 2]
    return np.asarray(fwd - gold, dtype=np.float32)[()]


def kernel(feats, transitions, mask, tags):
    feats = np.asarray(feats)
    transitions = np.asarray(transitions)
    mask = np.asarray(mask)
    tags = np.asarray(tags)

    if "nc" not in _CACHE:
        _CACHE["nc"] = _build_nc(debug=False)
    nc = _CACHE["nc"]

    from concourse import bass_utils

    in_maps = [
        _prep_core_inputs(feats, transitions, mask, tags, c) for c in range(NCORES)
    ]
    out = bass_utils.run_bass_kernel_spmd(nc, in_maps, core_ids=list(range(NCORES)))
    return _combine(out.results, mask)
